# revision 1
# baseline (speedup 1.0000x reference)
"""BoxFilter (9x9 box-sum, clamped borders) Trainium2 Bass kernel.

Input  x: [16, 3, 1024, 1024] f32, r=4 (hardcoded).
Output y: same shape; y[b,c,i,j] = sum of x[b,c,u,v] over the
(2r+1)x(2r+1) window centered at (i,j), clipped to the image bounds
(this is exactly what the reference's cumsum+diff computes).

Strategy (pure data parallel over 8 cores, 6 of the 48 images each):
  - H direction: banded 0/1 matmul on the TensorEngine. The image is
    processed in 9 overlapping 128-row slabs chosen so each output row
    block (120/124/60 rows) needs only rows inside one slab -> a single
    self-contained matmul per (slab, 512-col half), no PSUM
    accumulation across slabs.
  - W direction: single-pass running-window sum on the VectorEngine via
    tensor_tensor_scan: state = (y[t] + state) - y[t-9], which yields
    box_end[t] = sum_{k=max(0,t-8)}^{t} y[k]. Output row j (j<=W-r-1)
    is box_end[j+r]; the last r columns are fixed up with a tiny scan +
    one ScalarEngine activation.
  - PSUM -> SBUF copies ride on the ScalarEngine.
"""

import os
import numpy as np

from concourse import bass, mybir, tile, bacc
from concourse.bass_utils import run_bass_kernel_spmd

F32 = mybir.dt.float32
H, W = 1024, 1024
N_CORES = 8
IPC = 6  # images per core: (16*3)/8
R = 4
D = 2 * R + 1  # 9

# slabs: (row0, nrows, out0, nouts, band_col)
_SLABS = (
    [(0, 128, 0, 124, 0)]
    + [(120 * i, 128, 120 * i + 4, 120, 124) for i in range(1, 8)]
    + [(960, 64, 964, 60, 244)]
)
_BAND_COLS = 304  # 124 + 120 + 60


def _band_matrix() -> np.ndarray:
    bands = np.zeros((128, _BAND_COLS), np.float32)
    for row0, nrows, out0, nouts, bc in _SLABS[:1] + _SLABS[1:2] + _SLABS[8:]:
        for j in range(nouts):
            h_out = out0 + j
            lo = max(0, h_out - R) - row0
            hi = min(H - 1, h_out + R) - row0
            bands[lo : hi + 1, bc + j] = 1.0
    return bands


_CACHE: dict = {}

# Set by the most recent kernel() call (for test harnesses).
LAST_RESULTS = None


def _build():
    nc = bacc.Bacc(
        "TRN2", target_bir_lowering=False, debug=False, enable_asserts=False
    )
    x_d = nc.dram_tensor("x", [IPC, H, W], F32, kind="ExternalInput").ap()
    bands_d = nc.dram_tensor("bands", [128, _BAND_COLS], F32, kind="ExternalInput").ap()
    y_d = nc.dram_tensor("y", [IPC, H, W], F32, kind="ExternalOutput").ap()

    ADD = mybir.AluOpType.add
    SUB = mybir.AluOpType.subtract
    BYP = mybir.AluOpType.bypass

    with tile.TileContext(nc) as tc:
        with (
            tc.tile_pool(name="const", bufs=1) as const_pool,
            tc.tile_pool(name="xin", bufs=6) as in_pool,
            tc.tile_pool(name="ps", bufs=4, space="PSUM") as ps_pool,
            tc.tile_pool(name="yrow", bufs=4) as y_pool,
            tc.tile_pool(name="box", bufs=4) as box_pool,
        ):
            bands_t = const_pool.tile([128, _BAND_COLS], F32)
            nc.sync.dma_start(bands_t[:], bands_d[:])

            for img in range(IPC):
                for row0, nrows, out0, nouts, bc in _SLABS:
                    xs = in_pool.tile([128, W], F32, tag="xin")
                    nc.sync.dma_start(xs[:nrows], x_d[img, row0 : row0 + nrows, :])

                    ps = ps_pool.tile([128, W], F32, tag="ps")
                    for h in range(2):
                        nc.tensor.matmul(
                            ps[:nouts, h * 512 : (h + 1) * 512],
                            lhsT=bands_t[:nrows, bc : bc + nouts],
                            rhs=xs[:nrows, h * 512 : (h + 1) * 512],
                            start=True,
                            stop=True,
                        )

                    yt = y_pool.tile([128, W], F32, tag="yrow")
                    nc.scalar.copy(yt[:nouts, 0:512], ps[:nouts, 0:512])
                    nc.scalar.copy(yt[:nouts, 512:1024], ps[:nouts, 512:1024])

                    # bx layout: [0:1024] box_end, [1024:1028] right border
                    # output, [1028:1032] scratch cumsum.
                    bx = box_pool.tile([128, 1032], F32, tag="box")
                    # box_end[0..8] = cumsum of yt[0..8]
                    nc.vector.tensor_tensor_scan(
                        bx[:nouts, 0:D], yt[:nouts, 0:D], yt[:nouts, 0:D],
                        0.0, op0=ADD, op1=BYP,
                    )
                    # box_end[9..1023]: running window-9 sum
                    nc.vector.tensor_tensor_scan(
                        bx[:nouts, D:W], yt[:nouts, D:W], yt[:nouts, 0 : W - D],
                        bx[:nouts, D - 1 : D], op0=ADD, op1=SUB,
                    )
                    # scratch c[i] = cumsum of yt[W-9 .. W-6] (4 cols)
                    nc.vector.tensor_tensor_scan(
                        bx[:nouts, 1028:1032],
                        yt[:nouts, W - D : W - R - 1],
                        yt[:nouts, W - D : W - R - 1],
                        0.0, op0=ADD, op1=BYP,
                    )
                    # out[W-4+i] = box_end[W-1] - c[i]
                    nc.scalar.activation(
                        bx[:nouts, 1024:1028],
                        bx[:nouts, 1028:1032],
                        mybir.ActivationFunctionType.Identity,
                        bias=bx[:nouts, W - 1 : W],
                        scale=-1.0,
                    )
                    nc.sync.dma_start(
                        y_d[img, out0 : out0 + nouts, :], bx[:nouts, R : R + W]
                    )

    nc.compile()
    return nc


def kernel(x: np.ndarray, r) -> np.ndarray:
    global LAST_RESULTS
    x = np.asarray(x, dtype=np.float32)
    assert x.shape == (16, 3, H, W), x.shape
    assert int(r) == R, r

    nc = _CACHE.get("nc")
    if nc is None:
        nc = _CACHE["nc"] = _build()

    xr = np.ascontiguousarray(x.reshape(N_CORES, IPC, H, W))
    bands = _band_matrix()
    in_maps = [{"x": xr[c], "bands": bands} for c in range(N_CORES)]

    trace = bool(int(os.environ.get("BOX_TRACE", "0")))
    res = run_bass_kernel_spmd(
        nc, in_maps, list(range(N_CORES)), trace=trace
    )
    LAST_RESULTS = res
    y = np.stack([res.results[c]["y"] for c in range(N_CORES)])
    return y.reshape(16, 3, H, W)


# revision 2
# speedup vs baseline: 1.0055x; 1.0055x over previous
"""BoxFilter (9x9 box-sum, clamped borders) Trainium2 Bass kernel.

Input  x: [16, 3, 1024, 1024] f32, r=4 (hardcoded).
Output y: same shape; y[b,c,i,j] = sum of x[b,c,u,v] over the
(2r+1)x(2r+1) window centered at (i,j), clipped to the image bounds
(this is exactly what the reference's cumsum+diff computes).

Strategy (pure data parallel over 8 cores, 6 of the 48 images each):
  - H direction: banded 0/1 matmul on the TensorEngine. The image is
    processed in 9 overlapping 128-row slabs chosen so each output row
    block (120/124/60 rows) needs only rows inside one slab -> a single
    self-contained matmul per (slab, 512-col half), no PSUM
    accumulation across slabs.
  - W direction: single-pass running-window sum on the VectorEngine via
    tensor_tensor_scan: state = (y[t] + state) - y[t-9], which yields
    box_end[t] = sum_{k=max(0,t-8)}^{t} y[k]. Output row j (j<=W-r-1)
    is box_end[j+r]; the last r columns are fixed up with a tiny scan +
    one ScalarEngine activation.
  - PSUM -> SBUF copies ride on the ScalarEngine.
"""

import os
import numpy as np

from concourse import bass, mybir, tile, bacc
from concourse.bass_utils import run_bass_kernel_spmd

F32 = mybir.dt.float32
H, W = 1024, 1024
N_CORES = 8
IPC = 6  # images per core: (16*3)/8
R = 4
D = 2 * R + 1  # 9

# slabs: (row0, nrows, out0, nouts, band_col)
_SLABS = (
    [(0, 128, 0, 124, 0)]
    + [(120 * i, 128, 120 * i + 4, 120, 124) for i in range(1, 8)]
    + [(960, 64, 964, 60, 244)]
)
_BAND_COLS = 304  # 124 + 120 + 60


def _band_matrix() -> np.ndarray:
    bands = np.zeros((128, _BAND_COLS), np.float32)
    for row0, nrows, out0, nouts, bc in _SLABS[:1] + _SLABS[1:2] + _SLABS[8:]:
        for j in range(nouts):
            h_out = out0 + j
            lo = max(0, h_out - R) - row0
            hi = min(H - 1, h_out + R) - row0
            bands[lo : hi + 1, bc + j] = 1.0
    return bands


_CACHE: dict = {}

# Set by the most recent kernel() call (for test harnesses).
LAST_RESULTS = None


def _build():
    nc = bacc.Bacc(
        "TRN2", target_bir_lowering=False, debug=False, enable_asserts=False
    )
    x_d = nc.dram_tensor("x", [IPC, H, W], F32, kind="ExternalInput").ap()
    bands_d = nc.dram_tensor("bands", [128, _BAND_COLS], F32, kind="ExternalInput").ap()
    y_d = nc.dram_tensor("y", [IPC, H, W], F32, kind="ExternalOutput").ap()

    ADD = mybir.AluOpType.add
    SUB = mybir.AluOpType.subtract
    BYP = mybir.AluOpType.bypass

    with tile.TileContext(nc) as tc:
        with (
            tc.tile_pool(name="const", bufs=1) as const_pool,
            tc.tile_pool(name="xin", bufs=6) as in_pool,
            tc.tile_pool(name="ps", bufs=4, space="PSUM") as ps_pool,
            tc.tile_pool(name="yrow", bufs=4) as y_pool,
            tc.tile_pool(name="box", bufs=4) as box_pool,
        ):
            bands_t = const_pool.tile([128, _BAND_COLS], F32)
            nc.sync.dma_start(bands_t[:], bands_d[:])

            for img in range(IPC):
                for row0, nrows, out0, nouts, bc in _SLABS:
                    xs = in_pool.tile([128, W], F32, tag="xin")
                    nc.sync.dma_start(xs[:nrows], x_d[img, row0 : row0 + nrows, :])

                    ps = ps_pool.tile([128, W], F32, tag="ps")
                    for h in range(2):
                        nc.tensor.matmul(
                            ps[:nouts, h * 512 : (h + 1) * 512],
                            lhsT=bands_t[:nrows, bc : bc + nouts],
                            rhs=xs[:nrows, h * 512 : (h + 1) * 512],
                            start=True,
                            stop=True,
                        )

                    yt = y_pool.tile([128, W], F32, tag="yrow")
                    nc.scalar.copy(yt[:nouts, 0:512], ps[:nouts, 0:512])
                    nc.scalar.copy(yt[:nouts, 512:1024], ps[:nouts, 512:1024])

                    # bx layout: [0:1024] box_end, [1024:1028] right border
                    # output, [1028:1032] scratch cumsum.
                    bx = box_pool.tile([128, 1032], F32, tag="box")
                    # box_end[0..8] = cumsum of yt[0..8]
                    nc.vector.tensor_tensor_scan(
                        bx[:nouts, 0:D], yt[:nouts, 0:D], yt[:nouts, 0:D],
                        0.0, op0=ADD, op1=BYP,
                    )
                    # box_end[9..1023]: running window-9 sum
                    nc.vector.tensor_tensor_scan(
                        bx[:nouts, D:W], yt[:nouts, D:W], yt[:nouts, 0 : W - D],
                        bx[:nouts, D - 1 : D], op0=ADD, op1=SUB,
                    )
                    # scratch c[i] = cumsum of yt[W-9 .. W-6] (4 cols)
                    nc.vector.tensor_tensor_scan(
                        bx[:nouts, 1028:1032],
                        yt[:nouts, W - D : W - R - 1],
                        yt[:nouts, W - D : W - R - 1],
                        0.0, op0=ADD, op1=BYP,
                    )
                    # out[W-4+i] = box_end[W-1] - c[i]
                    nc.scalar.activation(
                        bx[:nouts, 1024:1028],
                        bx[:nouts, 1028:1032],
                        mybir.ActivationFunctionType.Identity,
                        bias=bx[:nouts, W - 1 : W],
                        scale=-1.0,
                    )
                    nc.sync.dma_start(
                        y_d[img, out0 : out0 + nouts, :], bx[:nouts, R : R + W]
                    )

    nc.compile()
    return nc


def kernel(x: np.ndarray, r) -> np.ndarray:
    global LAST_RESULTS
    x = np.asarray(x, dtype=np.float32)
    assert x.shape == (16, 3, H, W), x.shape
    assert int(r) == R, r

    nc = _CACHE.get("nc")
    if nc is None:
        nc = _CACHE["nc"] = _build()

    xr = np.ascontiguousarray(x.reshape(N_CORES, IPC, H, W))
    bands = _band_matrix()
    in_maps = [{"x": xr[c], "bands": bands} for c in range(N_CORES)]

    trace = bool(int(os.environ.get("BOX_TRACE", "0")))
    tmpdir = os.environ.get("BOX_TRACE_DIR") or None
    if tmpdir:
        os.makedirs(tmpdir, exist_ok=True)
    res = run_bass_kernel_spmd(
        nc, in_maps, list(range(N_CORES)), trace=trace, tmpdir=tmpdir
    )
    LAST_RESULTS = res
    y = np.stack([res.results[c]["y"] for c in range(N_CORES)])
    return y.reshape(16, 3, H, W)
